# Initial kernel scaffold
#
"""Mixtral-style GQA attention (B=1, S=2048, HID=4096, 32 q-heads / 8 kv-heads,
head_dim=128, NeoX RoPE, causal) on 8 Trainium2 NeuronCores.

Sharding: tensor-parallel over heads. Core i gets q-heads [4i..4i+3] and
kv-head i (w_qkv columns), plus the matching w_o rows. Each core computes a
full-shape partial of the output projection; the host sums the 8 partials
(the "all-reduce") and returns the full output.

Device layout notes:
 - All matmuls run in fp32r (TF32) at 1 cycle/row; every tensor consumed by an
   fp32r matmul is produced only by f32r-writing instructions (walrus checks).
 - hidden_states is passed pre-transposed (XT [HID, S]) so the QKV projection
   needs no on-device transpose: qkvT[f, s] = sum_h W[h, f] * XT[h, s].
 - Attention works in head-major [dim, seq] layout; only V is transposed
   on-device (16 PE transposes) to [seq, dim] for the PV matmul.
 - RoPE chunks are emitted inside the QKV loop right after each window's
   PSUM eviction, so the DVE rotates q/k while TensorE continues projecting.
 - Causal mask is applied additively (-3e4) on the score PSUM before exp;
   diagonal blocks only compute their live column range.
 - Softmax skips max-subtraction (scores are O(10), exp stays finite in f32)
   and normalizes after PV: attnT = (P^T V)^T * (1/rowsum) with rowsums from a
   ones-vector matmul accumulated alongside PV, broadcast via a rank-1 matmul.
 - o_proj is fully interleaved with attention, delayed by one q-window so its
   matmuls fill TensorE bubbles while ACT works through the exps.
 - 24 of 32 W k-tiles stay resident; 8 stream per window (frees SBUF so the
   whole w_o fits during the attention+o_proj phase).
"""
from contextlib import ExitStack

import numpy as np

import concourse.bacc as bacc
import concourse.tile as tile
from concourse import mybir
from concourse.bass_utils import run_bass_kernel_spmd

# ---- problem constants (hardcoded per contest contract) ----
HID = 4096
S = 2048
N_HEADS = 32
N_KV = 8
D = 128                    # head_dim
NCORES = 8
QH = N_HEADS // NCORES     # 4 q-heads per core
FEAT = QH * D + 2 * D      # 768 per-core qkv output columns (q0..q3, k, v)
FO = QH * D                # 512 per-core attn features for o_proj
ROPE_THETA = 10000.0
SCALE = D ** -0.5
MASK_NEG = -30000.0

P = 128
F32 = mybir.dt.float32
F32R = mybir.dt.float32r
EXP = mybir.ActivationFunctionType.Exp

NKT = HID // P     # 32 hidden k-tiles
NWRES = 24         # W k-tiles resident in SBUF; the rest stream per window
NSW = S // 512     # 4 seq windows
NM = FEAT // P     # 6 qkv m-tiles
NST = S // P       # 16 seq tiles

_CACHE = {}


def _build_nc():
    nc = bacc.Bacc("TRN2", target_bir_lowering=False, debug=False)

    xt = nc.dram_tensor("xt", [HID, S], F32R, kind="ExternalInput").ap()
    wqkv = nc.dram_tensor("wqkv", [HID, FEAT], F32R, kind="ExternalInput").ap()
    wo = nc.dram_tensor("wo", [FO, HID], F32R, kind="ExternalInput").ap()
    cos_d = nc.dram_tensor("cos", [D, S], F32, kind="ExternalInput").ap()
    sinr_d = nc.dram_tensor("sinr", [D, S], F32, kind="ExternalInput").ap()
    mask_d = nc.dram_tensor("masks", [P, 4, 512], F32, kind="ExternalInput").ap()
    ones_d = nc.dram_tensor("ones_col", [P, 1], F32R, kind="ExternalInput").ap()
    onesr_d = nc.dram_tensor("ones_row", [1, P], F32R, kind="ExternalInput").ap()
    id_d = nc.dram_tensor("ident", [P, P], F32, kind="ExternalInput").ap()
    out = nc.dram_tensor("out", [S, HID], F32, kind="ExternalOutput").ap()

    with tile.TileContext(nc) as tc:
        _kernel(tc, xt, wqkv, wo, cos_d, sinr_d, mask_d, ones_d, onesr_d, id_d, out)
    nc.compile()
    return nc


def _kernel(tc, xt, wqkv, wo, cos_d, sinr_d, mask_d, ones_d, onesr_d, id_d, out):
    nc = tc.nc

    with ExitStack() as big:
        persist = big.enter_context(tc.tile_pool(name="persist", bufs=1))
        ones_sb = persist.tile([P, 1], F32R)
        onesr_sb = persist.tile([1, P], F32R)
        id_sb = persist.tile([P, P], F32)
        nc.sync.dma_start(out=ones_sb, in_=ones_d)
        nc.sync.dma_start(out=onesr_sb, in_=onesr_d)
        nc.sync.dma_start(out=id_sb, in_=id_d)
        roped = persist.tile([P, QH + 1, S], F32R)  # rotated q0..q3, K
        v_nat = persist.tile([P, NST, D], F32R)     # V in [seq-tile, dim] blocks

        # ---- phase 1: qkvT = wqkv^T @ XT, rope chunks interleaved ----
        wq3 = wqkv.rearrange("(kt p) f -> p kt f", p=P)
        with tc.tile_pool(name="qkvf", bufs=1) as qkvf_pool, \
             tc.tile_pool(name="cs", bufs=1) as cs_pool, \
             tc.tile_pool(name="rtmp", bufs=2) as rtmp, \
             tc.tile_pool(name="wq", bufs=1) as wq_pool, \
             tc.tile_pool(name="ws", bufs=3) as wstream, \
             tc.tile_pool(name="xts", bufs=4) as xt_pool:
            qkv = qkvf_pool.tile([P, NM, S], F32)   # raw qkvT (pre-rope)
            cos_sb = cs_pool.tile([D, S], F32)
            sinr_sb = cs_pool.tile([D, S], F32)
            nc.sync.dma_start(out=cos_sb, in_=cos_d)
            nc.sync.dma_start(out=sinr_sb, in_=sinr_d)

            def rope_chunk(m, sw):
                """roped[:,m,win] = qkv*cos + rot_half(qkv)*sin over a window."""
                win = slice(sw * 512, (sw + 1) * 512)
                row = qkv[:, m, win]
                tmp = rtmp.tile([P, 512], F32, tag="rt", name="ropetmp")
                nc.vector.tensor_mul(tmp[0:64, :], row[64:128, :],
                                     sinr_sb[64:128, win])
                nc.vector.tensor_mul(tmp[64:128, :], row[0:64, :],
                                     sinr_sb[0:64, win])
                nc.vector.tensor_mul(row, row, cos_sb[:, win])
                nc.vector.tensor_add(roped[:, m, win], row, tmp)

            w_res = [wq_pool.tile([P, FEAT], F32R, tag=f"w{kt}", name=f"w{kt}")
                     for kt in range(NWRES)]
            for kt in range(NWRES):
                nc.sync.dma_start(out=w_res[kt], in_=wq3[:, kt, :])
            with tc.tile_pool(name="qkps", bufs=1, space="PSUM") as qk_ps, \
                 tc.tile_pool(name="tps", bufs=2, space="PSUM") as tp_ps:
                for sw in range(NSW):
                    ps = [qk_ps.tile([P, 512], F32, tag=f"m{m}",
                                     name=f"qkps{m}") for m in range(NM)]
                    for kt in range(NKT):
                        if kt < NWRES:
                            wt = w_res[kt]
                        else:
                            wt = wstream.tile([P, FEAT], F32R, tag="ws",
                                              name="wstrm")
                            nc.sync.dma_start(out=wt, in_=wq3[:, kt, :])
                        xts = xt_pool.tile([P, 512], F32R)
                        nc.scalar.dma_start(
                            out=xts,
                            in_=xt[kt * P:(kt + 1) * P, sw * 512:(sw + 1) * 512])
                        for m in range(NM):
                            nc.tensor.matmul(
                                ps[m], wt[:, m * P:(m + 1) * P], xts,
                                start=(kt == 0), stop=(kt == NKT - 1))
                    for m in range(NM):
                        nc.scalar.copy(out=qkv[:, m, sw * 512:(sw + 1) * 512],
                                       in_=ps[m])
                    for m in range(QH + 1):   # rope q0..q3 + K for this window
                        rope_chunk(m, sw)
                    # V transpose for this window's 4 seq blocks
                    for st in range(4 * sw, 4 * sw + 4):
                        tp = tp_ps.tile([P, P], F32, tag="tp", name="tpps")
                        nc.tensor.transpose(
                            tp, qkv[:, QH + 1, st * P:(st + 1) * P], id_sb)
                        nc.vector.tensor_copy(out=v_nat[:, st, :], in_=tp)

        # ---- phase 3: attention with o_proj fully interleaved (1-qw delay) ----
        kt_row = roped[:, QH, :]
        wo3 = wo.rearrange("(ft p) e -> p ft e", p=P)
        with tc.tile_pool(name="atn", bufs=1) as atn_pool, \
             tc.tile_pool(name="wop", bufs=1) as wo_pool, \
             tc.tile_pool(name="mskp", bufs=1) as mask_pool, \
             tc.tile_pool(name="pts", bufs=10) as pt_pool, \
             tc.tile_pool(name="nsc", bufs=2) as norm_sc, \
             tc.tile_pool(name="ost", bufs=6) as o_stage, \
             tc.tile_pool(name="stps", bufs=2, space="PSUM") as st_ps, \
             tc.tile_pool(name="pvps", bufs=2, space="PSUM") as pv_ps, \
             tc.tile_pool(name="smps", bufs=2, space="PSUM") as sum_ps, \
             tc.tile_pool(name="ops", bufs=1, space="PSUM") as o_ps:
            attnT = atn_pool.tile([P, QH, S], F32R)
            wo_sb = wo_pool.tile([P, QH, HID], F32R)
            mask_sb = mask_pool.tile([P, 4, 512], F32)
            nc.sync.dma_start(out=mask_sb, in_=mask_d)
            for f in range(QH):
                nc.sync.dma_start(out=wo_sb[:, f, :], in_=wo3[:, f, :])

            def oproj(st):
                # 2 PSUM banks per group (PSUM budget: 2+2+2+2 = 8)
                ss = slice(st * P, (st + 1) * P)
                for ecg in range(4):
                    ops = [o_ps.tile([P, 512], F32, tag=f"o{ec}",
                                     name=f"ops{ec}") for ec in range(2)]
                    for f in range(QH):
                        for ec in range(2):
                            c0 = (ecg * 2 + ec) * 512
                            nc.tensor.matmul(
                                ops[ec], attnT[:, f, ss],
                                wo_sb[:, f, c0:c0 + 512],
                                start=(f == 0), stop=(f == QH - 1))
                    for ec in range(2):
                        c0 = (ecg * 2 + ec) * 512
                        stg = o_stage.tile([P, 512], F32, tag="stg",
                                           name="ostg")
                        nc.vector.tensor_copy(out=stg, in_=ops[ec])
                        nc.scalar.dma_start(out=out[ss, c0:c0 + 512], in_=stg)

            order = list(range(NSW))
            for idx, qw in enumerate(order):
                n_kt = 4 * (qw + 1)
                qs = slice(qw * 512, (qw + 1) * 512)
                for h in range(QH):
                    qr = roped[:, h, qs]
                    pv = pv_ps.tile([P, 512], F32, tag="pv", name="pvps")
                    sm = sum_ps.tile([1, 512], F32, tag="sm", name="smps")
                    for kt in range(n_kt):
                        j = kt - 4 * qw
                        # diagonal blocks: columns < 128j are fully masked;
                        # never compute or read them.
                        c0 = max(j, 0) * P
                        cs_ = slice(c0, 512)
                        stp = st_ps.tile([P, 512], F32, tag="st", name="stps")
                        nc.tensor.matmul(
                            stp[:, cs_], kt_row[:, kt * P:(kt + 1) * P],
                            qr[:, cs_], start=True, stop=True)
                        if j >= 0:   # diagonal: additive causal mask
                            nc.vector.tensor_add(stp[:, cs_], stp[:, cs_],
                                                 mask_sb[:, j, cs_])
                        pt = pt_pool.tile([P, 512], F32R, tag="pt",
                                          name="ptile")
                        nc.scalar.activation(out=pt[:, cs_], in_=stp[:, cs_],
                                             func=EXP, scale=SCALE)
                        nc.tensor.matmul(pv[:, cs_], v_nat[:, kt, :],
                                         pt[:, cs_], start=(kt == 0),
                                         stop=(kt == n_kt - 1))
                        nc.tensor.matmul(sm[:, cs_], ones_sb, pt[:, cs_],
                                         start=(kt == 0),
                                         stop=(kt == n_kt - 1))
                    # normalize via rank-1 broadcast of the rowsums
                    sm_sb = norm_sc.tile([1, 512], F32R, tag="sms", name="smsb")
                    nc.scalar.copy(out=sm_sb, in_=sm)
                    bc = st_ps.tile([P, 512], F32, tag="st", name="bcps")
                    nc.tensor.matmul(bc, onesr_sb, sm_sb, start=True, stop=True)
                    rec = norm_sc.tile([P, 512], F32, tag="rec", name="recb")
                    nc.vector.reciprocal_approx_fast(out=rec, in_=bc)
                    nc.vector.tensor_mul(attnT[:, h, qs], pv, rec)

                if idx > 0:   # o_proj for the previous window (bubble filler)
                    for st in range(4 * order[idx - 1], 4 * order[idx - 1] + 4):
                        oproj(st)
            for st in range(4 * order[-1], 4 * order[-1] + 4):
                oproj(st)


def _host_inputs(positions, hidden_states, w_qkv, w_o):
    """Build the 8 per-core input maps (host-side sharding + layout prep)."""
    x = np.ascontiguousarray(hidden_states.reshape(S, HID).T)  # [HID, S]

    pos = positions.reshape(S).astype(np.float32)
    inv = (1.0 / (ROPE_THETA ** (np.arange(0, D, 2, dtype=np.float32) / D)))
    ang = inv[:, None] * pos[None, :]                      # [64, S]
    cos = np.concatenate([np.cos(ang), np.cos(ang)], 0).astype(np.float32)
    sinr = np.concatenate([np.sin(ang), -np.sin(ang)], 0).astype(np.float32)

    masks = np.zeros((P, 4, 512), dtype=np.float32)
    k_loc = np.arange(P)[:, None]
    q_loc = np.arange(512)[None, :]
    for j in range(4):
        masks[:, j, :] = np.where(k_loc + P * j <= q_loc, 0.0, MASK_NEG)

    ones_col = np.ones((P, 1), np.float32)
    ones_row = np.ones((1, P), np.float32)
    ident = np.eye(P, dtype=np.float32)

    in_maps = []
    for i in range(NCORES):
        wq = w_qkv[:, i * FO:(i + 1) * FO]
        wk = w_qkv[:, N_HEADS * D + i * D: N_HEADS * D + (i + 1) * D]
        wv = w_qkv[:, (N_HEADS + N_KV) * D + i * D:
                   (N_HEADS + N_KV) * D + (i + 1) * D]
        wqkv_i = np.ascontiguousarray(np.concatenate([wq, wk, wv], axis=1))
        wo_i = np.ascontiguousarray(w_o[i * FO:(i + 1) * FO, :])
        in_maps.append({
            "xt": x, "wqkv": wqkv_i, "wo": wo_i, "cos": cos, "sinr": sinr,
            "masks": masks, "ones_col": ones_col, "ones_row": ones_row,
            "ident": ident,
        })
    return in_maps


def kernel(positions, hidden_states, w_qkv, w_o):
    positions = np.asarray(positions)
    hidden_states = np.asarray(hidden_states, dtype=np.float32)
    w_qkv = np.asarray(w_qkv, dtype=np.float32)
    w_o = np.asarray(w_o, dtype=np.float32)

    if "nc" not in _CACHE:
        _CACHE["nc"] = _build_nc()
    nc = _CACHE["nc"]

    in_maps = _host_inputs(positions, hidden_states, w_qkv, w_o)
    res = run_bass_kernel_spmd(nc, in_maps, list(range(NCORES)))

    acc = np.zeros((S, HID), dtype=np.float32)
    for c in range(NCORES):
        acc += res.results[c]["out"]
    return acc.reshape(1, S, HID)



# revision 9
# speedup vs baseline: 1.4484x; 1.4484x over previous
"""Mixtral-style GQA attention (B=1, S=2048, HID=4096, 32 q-heads / 8 kv-heads,
head_dim=128, NeoX RoPE, causal) on 8 Trainium2 NeuronCores.

Sharding: tensor-parallel over heads. Core i gets q-heads [4i..4i+3] and
kv-head i (w_qkv columns), plus the matching w_o rows. Each core computes a
full-shape fp16 partial of the output projection; the host sums the 8
partials in fp32 (the "all-reduce") and returns the full output.

Device layout (v4, all-fp16):
 - Every matmul operand is fp16 (fp32r measures ~2 cycles/row on TRN2;
   16-bit streams 1 col/cycle and FWL hides LDWEIGHTS). PSUM stays fp32.
 - Softmax skips max-subtraction but shifts exp by a constant -5 so P and
   its rowsums fit fp16 comfortably (max score*SCALE ~ 10, rowsums < 256;
   the shift cancels exactly in P/rowsum). Causal mask is a 0/1 fp16
   multiply on the exp output, off the score->exp critical path.
 - Rowsums accumulate on DVE in fp16 (2x mode) and broadcast to 128
   partitions with one all-ones [128,128] matmul per head-window; no
   per-k-tile rowsum matmuls on the PE.
 - PV and rowsum consumption lag the score/exp pipeline by one k-tile so
   the in-order PE queue never waits on ACT exp latency.
 - Attention windows run 3,2,1,0 (legal: all K/V ready after phase 1) so
   the first window is the densest and the HAM clock gate never drops.
 - o_proj (previous window) interleaves into the attention k-tile loop at
   micro-op granularity; 4 PSUM banks, staging evictions alternate
   DVE/ACT into a [128, 4096] fp16 tile -> one output DMA per seq-tile.
 - hidden_states passed pre-transposed (XT [HID, S]); w_qkv fully
   SBUF-resident; XT streams per-window in batched [128, 8, 512] DMAs
   issued from the Sync engine.
"""
from contextlib import ExitStack

import numpy as np

import concourse.bacc as bacc
import concourse.tile as tile
from concourse import mybir
from concourse.bass_utils import run_bass_kernel_spmd

# ---- problem constants (hardcoded per contest contract) ----
HID = 4096
S = 2048
N_HEADS = 32
N_KV = 8
D = 128                    # head_dim
NCORES = 8
QH = N_HEADS // NCORES     # 4 q-heads per core
FEAT = QH * D + 2 * D      # 768 per-core qkv output columns (q0..q3, k, v)
FO = QH * D                # 512 per-core attn features for o_proj
ROPE_THETA = 10000.0
SCALE = D ** -0.5
ESHIFT = -5.0              # exp(s*SCALE + ESHIFT); cancels in normalization

P = 128
F32 = mybir.dt.float32
F16 = mybir.dt.float16
EXP = mybir.ActivationFunctionType.Exp

NKT = HID // P     # 32 hidden k-tiles
NSW = S // 512     # 4 seq windows
NM = FEAT // P     # 6 qkv m-tiles
NST = S // P       # 16 seq tiles

_CACHE = {}


def _build_nc():
    nc = bacc.Bacc("TRN2", target_bir_lowering=False, debug=False)

    xt = nc.dram_tensor("xt", [HID, S], F16, kind="ExternalInput").ap()
    wqkv = nc.dram_tensor("wqkv", [HID, FEAT], F16, kind="ExternalInput").ap()
    wo = nc.dram_tensor("wo", [FO, HID], F16, kind="ExternalInput").ap()
    cos_d = nc.dram_tensor("cos", [D, S], F16, kind="ExternalInput").ap()
    sinr_d = nc.dram_tensor("sinr", [D, S], F16, kind="ExternalInput").ap()
    zmask_d = nc.dram_tensor("zmask", [P, 4, 512], F16, kind="ExternalInput").ap()
    ones_d = nc.dram_tensor("ones_sq", [P, P], F16, kind="ExternalInput").ap()
    ebias_d = nc.dram_tensor("ebias", [P, 1], F32, kind="ExternalInput").ap()
    id_d = nc.dram_tensor("ident", [P, P], F16, kind="ExternalInput").ap()
    out = nc.dram_tensor("out", [S, HID], F16, kind="ExternalOutput").ap()

    with tile.TileContext(nc) as tc:
        _kernel(tc, xt, wqkv, wo, cos_d, sinr_d, zmask_d, ones_d, ebias_d,
                id_d, out)
    nc.compile()
    return nc


def _kernel(tc, xt, wqkv, wo, cos_d, sinr_d, zmask_d, ones_d, ebias_d, id_d,
            out):
    nc = tc.nc

    with ExitStack() as big:
        persist = big.enter_context(tc.tile_pool(name="persist", bufs=1))
        ones_sb = persist.tile([P, P], F16)
        ebias_sb = persist.tile([P, 1], F32)
        id_sb = persist.tile([P, P], F16)
        roped = persist.tile([P, QH + 1, S], F16)   # rotated q0..q3, K
        v_nat = persist.tile([P, NST, D], F16)      # V in [seq-tile, dim]
        w_sb = persist.tile([P, NKT, FEAT], F16)    # full w_qkv resident
        wq3 = wqkv.rearrange("(kt p) f -> p kt f", p=P)
        xt3 = xt.rearrange("(kt p) s -> p kt s", p=P)

        # ---- phase 1: qkvT = wqkv^T @ XT, rope chunks interleaved ----
        with tc.tile_pool(name="qkvf", bufs=2) as qkvf_pool, \
             tc.tile_pool(name="cs", bufs=1) as cs_pool, \
             tc.tile_pool(name="rtmp", bufs=2) as rtmp, \
             tc.tile_pool(name="xts", bufs=4) as xt_pool:
            cos_sb = cs_pool.tile([D, S], F16)
            sinr_sb = cs_pool.tile([D, S], F16)

            # startup: first matmul only needs w k-tiles 0-1 + xt chunk 0
            # (~1.6 MB); order the boot DMAs accordingly.
            xts0 = []
            nc.sync.dma_start(out=w_sb[:, 0:2, :], in_=wq3[:, 0:2, :])
            t = xt_pool.tile([P, 8, 512], F16, tag="xt", name="xtchunk")
            nc.sync.dma_start(out=t, in_=xt3[:, 0:8, 0:512])
            xts0.append(t)
            nc.sync.dma_start(out=w_sb[:, 2:8, :], in_=wq3[:, 2:8, :])
            for c in range(1, 4):
                t = xt_pool.tile([P, 8, 512], F16, tag="xt", name="xtchunk")
                nc.sync.dma_start(out=t, in_=xt3[:, 8 * c:8 * c + 8, 0:512])
                xts0.append(t)
                nc.sync.dma_start(out=w_sb[:, 8 * c:8 * c + 8, :],
                                  in_=wq3[:, 8 * c:8 * c + 8, :])
            nc.sync.dma_start(out=cos_sb, in_=cos_d)
            nc.sync.dma_start(out=sinr_sb, in_=sinr_d)
            nc.sync.dma_start(out=ones_sb, in_=ones_d)
            nc.sync.dma_start(out=ebias_sb, in_=ebias_d)
            nc.sync.dma_start(out=id_sb, in_=id_d)

            with tc.tile_pool(name="qkps", bufs=1, space="PSUM") as qk_ps, \
                 tc.tile_pool(name="tps", bufs=2, space="PSUM") as tp_ps:
                for sw in range(NSW):
                    win = slice(sw * 512, (sw + 1) * 512)
                    qkv = qkvf_pool.tile([P, NM, 512], F16, tag="qkv",
                                         name="qkvwin")
                    ps = [qk_ps.tile([P, 512], F32, tag=f"m{m}",
                                     name=f"qkps{m}") for m in range(NM)]
                    if sw == 0:
                        xts = xts0
                    else:
                        xts = []
                        for c in range(4):
                            t = xt_pool.tile([P, 8, 512], F16, tag="xt",
                                             name="xtchunk")
                            nc.sync.dma_start(
                                out=t, in_=xt3[:, 8 * c:8 * c + 8, win])
                            xts.append(t)
                    for kt in range(NKT):
                        for m in range(NM):
                            nc.tensor.matmul(
                                ps[m], w_sb[:, kt, m * P:(m + 1) * P],
                                xts[kt // 8][:, kt % 8, :],
                                start=(kt == 0), stop=(kt == NKT - 1))
                    for m in range(NM):
                        nc.scalar.copy(out=qkv[:, m, :], in_=ps[m])
                    for m in range(QH + 1):   # rope q0..q3 + K, this window
                        row = qkv[:, m, :]
                        tmp = rtmp.tile([P, 512], F16, tag="rt",
                                        name="ropetmp")
                        nc.vector.tensor_mul(tmp[0:64, :], row[64:128, :],
                                             sinr_sb[64:128, win])
                        nc.vector.tensor_mul(tmp[64:128, :], row[0:64, :],
                                             sinr_sb[0:64, win])
                        nc.vector.tensor_mul(row, row, cos_sb[:, win])
                        nc.vector.tensor_add(roped[:, m, win], row, tmp)
                    # V transpose for this window's 4 seq blocks
                    for st in range(4 * sw, 4 * sw + 4):
                        tp = tp_ps.tile([P, P], F16, tag="tp", name="tpps")
                        nc.tensor.transpose(
                            tp, qkv[:, QH + 1, (st % 4) * P:(st % 4 + 1) * P],
                            id_sb)
                        nc.vector.tensor_copy(out=v_nat[:, st, :], in_=tp)

        # ---- phase 2: attention with o_proj interleaved per k-tile ----
        kt_row = roped[:, QH, :]
        wo3 = wo.rearrange("(ft p) e -> p ft e", p=P)
        with tc.tile_pool(name="atn", bufs=1) as atn_pool, \
             tc.tile_pool(name="wop", bufs=1) as wo_pool, \
             tc.tile_pool(name="mskp", bufs=1) as mask_pool, \
             tc.tile_pool(name="pts", bufs=8) as pt_pool, \
             tc.tile_pool(name="accp", bufs=2) as acc_pool, \
             tc.tile_pool(name="nsc", bufs=2) as norm_sc, \
             tc.tile_pool(name="ost", bufs=2) as o_stage, \
             tc.tile_pool(name="stps", bufs=2, space="PSUM") as st_ps, \
             tc.tile_pool(name="pvps", bufs=2, space="PSUM") as pv_ps, \
             tc.tile_pool(name="ops", bufs=1, space="PSUM") as o_ps:
            attnT = atn_pool.tile([P, QH, S], F16)
            wo_sb = wo_pool.tile([P, QH, HID], F16)
            zmask_sb = mask_pool.tile([P, 4, 512], F16)
            nc.sync.dma_start(out=zmask_sb, in_=zmask_d)
            for f in range(QH):
                nc.sync.dma_start(out=wo_sb[:, f, :], in_=wo3[:, f, :])

            def make_oproj_thunks(st):
                """o_proj for seq-tile st as a list of single-step thunks.
                4 PSUM banks: ecg pairs alternate (o0,o1)/(o2,o3)."""
                ss = slice(st * P, (st + 1) * P)
                state = {}
                thunks = []

                def alloc_stage():
                    state['stg'] = o_stage.tile([P, HID], F16, tag="stg",
                                                name="ostg")
                for ecg in range(4):
                    for f in range(QH):
                        for ec in range(2):
                            def mm(f=f, ec=ec, ecg=ecg):
                                if f == 0 and ec == 0:
                                    if ecg == 0:
                                        alloc_stage()
                                    b = 2 * (ecg % 2)
                                    state[ecg] = [
                                        o_ps.tile([P, 512], F32,
                                                  tag=f"o{b + e}",
                                                  name=f"ops{b + e}")
                                        for e in range(2)]
                                c0 = (ecg * 2 + ec) * 512
                                nc.tensor.matmul(
                                    state[ecg][ec], attnT[:, f, ss],
                                    wo_sb[:, f, c0:c0 + 512],
                                    start=(f == 0), stop=(f == QH - 1))
                            thunks.append(mm)
                    for ec in range(2):
                        def stage(ec=ec, ecg=ecg):
                            c0 = (ecg * 2 + ec) * 512
                            dst = state['stg'][:, c0:c0 + 512]
                            if ec == 0:
                                nc.vector.tensor_copy(out=dst,
                                                      in_=state[ecg][ec])
                            else:
                                nc.scalar.copy(out=dst, in_=state[ecg][ec])
                        thunks.append(stage)

                def dma():
                    nc.sync.dma_start(out=out[ss, :], in_=state['stg'])
                thunks.append(dma)
                return thunks

            pending = []          # o_proj thunks of previous windows

            def drain(k):
                for _ in range(min(k, len(pending))):
                    pending.pop(0)()

            for qw in reversed(range(NSW)):   # densest window first (HAM)
                n_kt = 4 * (qw + 1)
                qs = slice(qw * 512, (qw + 1) * 512)
                iters_left = QH * n_kt
                for h in range(QH):
                    qr = roped[:, h, qs]
                    pv = pv_ps.tile([P, 512], F32, tag="pv", name="pvps")
                    acc = acc_pool.tile([P, 512], F16, tag="acc", name="acct")
                    prev = None
                    for kt in range(n_kt):
                        j = kt - 4 * qw
                        c0 = max(j, 0) * P
                        cs_ = slice(c0, 512)
                        stp = st_ps.tile([P, 512], F32, tag="st", name="stps")
                        nc.tensor.matmul(
                            stp[:, cs_], kt_row[:, kt * P:(kt + 1) * P],
                            qr[:, cs_], start=True, stop=True)
                        pt = pt_pool.tile([P, 512], F16, tag="pt",
                                          name="ptile")
                        nc.scalar.activation(out=pt[:, cs_], in_=stp[:, cs_],
                                             func=EXP, scale=SCALE,
                                             bias=ebias_sb)
                        if j >= 0:   # diagonal: zero the masked region
                            nc.vector.tensor_mul(pt[:, cs_], pt[:, cs_],
                                                 zmask_sb[:, j, cs_])
                        if kt == 0:
                            nc.vector.tensor_copy(out=acc, in_=pt)
                        else:
                            nc.vector.tensor_add(acc[:, cs_], acc[:, cs_],
                                                 pt[:, cs_])
                        # PV lags one k-tile so the PE never waits on exp
                        if prev is not None:
                            pk, pp, pcs = prev
                            nc.tensor.matmul(pv[:, pcs], v_nat[:, pk, :],
                                             pp[:, pcs], start=(pk == 0),
                                             stop=False)
                        if pending:
                            drain(-(-len(pending) // iters_left))
                        iters_left -= 1
                        prev = (kt, pt, cs_)
                    pk, pp, pcs = prev
                    nc.tensor.matmul(pv[:, pcs], v_nat[:, pk, :], pp[:, pcs],
                                     start=(pk == 0), stop=True)
                    drain(2)
                    # rowsum broadcast: bc[m, q] = sum_p acc[p, q]
                    bc = st_ps.tile([P, 512], F32, tag="st", name="bcps")
                    nc.tensor.matmul(bc, ones_sb, acc, start=True, stop=True)
                    rec = norm_sc.tile([P, 512], F32, tag="rec", name="recb")
                    nc.vector.reciprocal_approx_fast(out=rec, in_=bc)
                    nc.vector.tensor_mul(attnT[:, h, qs], pv, rec)

                for st in range(4 * qw, 4 * qw + 4):
                    pending.extend(make_oproj_thunks(st))
            drain(len(pending))


def _host_inputs(positions, hidden_states, w_qkv, w_o):
    """Build the 8 per-core input maps (host-side sharding + layout prep)."""
    f16 = np.float16
    x = np.ascontiguousarray(
        hidden_states.reshape(S, HID).T.astype(f16))     # [HID, S] fp16

    pos = positions.reshape(S).astype(np.float32)
    inv = (1.0 / (ROPE_THETA ** (np.arange(0, D, 2, dtype=np.float32) / D)))
    ang = inv[:, None] * pos[None, :]                    # [64, S]
    cos = np.concatenate([np.cos(ang), np.cos(ang)], 0).astype(f16)
    sinr = np.concatenate([np.sin(ang), -np.sin(ang)], 0).astype(f16)

    zmask = np.zeros((P, 4, 512), dtype=f16)
    k_loc = np.arange(P)[:, None]
    q_loc = np.arange(512)[None, :]
    for j in range(4):
        zmask[:, j, :] = (k_loc + P * j <= q_loc).astype(f16)

    ones_sq = np.ones((P, P), f16)
    ebias = np.full((P, 1), ESHIFT, np.float32)
    ident = np.eye(P, dtype=f16)

    in_maps = []
    for i in range(NCORES):
        wq = w_qkv[:, i * FO:(i + 1) * FO]
        wk = w_qkv[:, N_HEADS * D + i * D: N_HEADS * D + (i + 1) * D]
        wv = w_qkv[:, (N_HEADS + N_KV) * D + i * D:
                   (N_HEADS + N_KV) * D + (i + 1) * D]
        wqkv_i = np.ascontiguousarray(
            np.concatenate([wq, wk, wv], axis=1).astype(f16))
        wo_i = np.ascontiguousarray(w_o[i * FO:(i + 1) * FO, :].astype(f16))
        in_maps.append({
            "xt": x, "wqkv": wqkv_i, "wo": wo_i, "cos": cos, "sinr": sinr,
            "zmask": zmask, "ones_sq": ones_sq, "ebias": ebias,
            "ident": ident,
        })
    return in_maps


def kernel(positions, hidden_states, w_qkv, w_o):
    positions = np.asarray(positions)
    hidden_states = np.asarray(hidden_states, dtype=np.float32)
    w_qkv = np.asarray(w_qkv, dtype=np.float32)
    w_o = np.asarray(w_o, dtype=np.float32)

    if "nc" not in _CACHE:
        _CACHE["nc"] = _build_nc()
    nc = _CACHE["nc"]

    in_maps = _host_inputs(positions, hidden_states, w_qkv, w_o)
    res = run_bass_kernel_spmd(nc, in_maps, list(range(NCORES)))
    _CACHE["last_res"] = res

    acc = np.zeros((S, HID), dtype=np.float32)
    for c in range(NCORES):
        acc += res.results[c]["out"].astype(np.float32)
    return acc.reshape(1, S, HID)


# revision 12
# speedup vs baseline: 1.4664x; 1.0125x over previous
"""Mixtral-style GQA attention (B=1, S=2048, HID=4096, 32 q-heads / 8 kv-heads,
head_dim=128, NeoX RoPE, causal) on 8 Trainium2 NeuronCores.

Sharding: tensor-parallel over heads. Core i gets q-heads [4i..4i+3] and
kv-head i (w_qkv columns), plus the matching w_o rows. Each core computes a
full-shape fp16 partial of the output projection; the host sums the 8
partials in fp32 (the "all-reduce") and returns the full output.

Device layout (v4, all-fp16):
 - Every matmul operand is fp16 (fp32r measures ~2 cycles/row on TRN2;
   16-bit streams 1 col/cycle and FWL hides LDWEIGHTS). PSUM stays fp32.
 - Softmax skips max-subtraction but shifts exp by a constant -5 so P and
   its rowsums fit fp16 comfortably (max score*SCALE ~ 10, rowsums < 256;
   the shift cancels exactly in P/rowsum). Causal mask is a 0/1 fp16
   multiply on the exp output, off the score->exp critical path.
 - Rowsums accumulate on DVE in fp16 (2x mode) and broadcast to 128
   partitions with one all-ones [128,128] matmul per head-window; no
   per-k-tile rowsum matmuls on the PE.
 - PV and rowsum consumption lag the score/exp pipeline by one k-tile so
   the in-order PE queue never waits on ACT exp latency.
 - Attention windows run 3,2,1,0 (legal: all K/V ready after phase 1) so
   the first window is the densest and the HAM clock gate never drops.
 - o_proj (previous window) interleaves into the attention k-tile loop at
   micro-op granularity; 4 PSUM banks, staging evictions alternate
   DVE/ACT into a [128, 4096] fp16 tile -> one output DMA per seq-tile.
 - hidden_states passed pre-transposed (XT [HID, S]); w_qkv fully
   SBUF-resident; XT streams per-window in batched [128, 8, 512] DMAs
   issued from the Sync engine.
"""
from contextlib import ExitStack

import numpy as np

import concourse.bacc as bacc
import concourse.tile as tile
from concourse import mybir
from concourse.bass_utils import run_bass_kernel_spmd

# ---- problem constants (hardcoded per contest contract) ----
HID = 4096
S = 2048
N_HEADS = 32
N_KV = 8
D = 128                    # head_dim
NCORES = 8
QH = N_HEADS // NCORES     # 4 q-heads per core
FEAT = QH * D + 2 * D      # 768 per-core qkv output columns (q0..q3, k, v)
FO = QH * D                # 512 per-core attn features for o_proj
ROPE_THETA = 10000.0
SCALE = D ** -0.5
ESHIFT = -5.0              # exp(s*SCALE + ESHIFT); cancels in normalization

P = 128
F32 = mybir.dt.float32
F16 = mybir.dt.float16
EXP = mybir.ActivationFunctionType.Exp

NKT = HID // P     # 32 hidden k-tiles
NSW = S // 512     # 4 seq windows
NM = FEAT // P     # 6 qkv m-tiles
NST = S // P       # 16 seq tiles

_CACHE = {}


def _build_nc():
    nc = bacc.Bacc("TRN2", target_bir_lowering=False, debug=False)

    xt = nc.dram_tensor("xt", [HID, S], F16, kind="ExternalInput").ap()
    wqkv = nc.dram_tensor("wqkv", [HID, FEAT], F16, kind="ExternalInput").ap()
    wo = nc.dram_tensor("wo", [FO, HID], F16, kind="ExternalInput").ap()
    cos_d = nc.dram_tensor("cos", [D, S], F16, kind="ExternalInput").ap()
    sinr_d = nc.dram_tensor("sinr", [D, S], F16, kind="ExternalInput").ap()
    zmask_d = nc.dram_tensor("zmask", [P, 4, 512], F16, kind="ExternalInput").ap()
    ones_d = nc.dram_tensor("ones_sq", [P, P], F16, kind="ExternalInput").ap()
    ebias_d = nc.dram_tensor("ebias", [P, 1], F32, kind="ExternalInput").ap()
    id_d = nc.dram_tensor("ident", [P, P], F16, kind="ExternalInput").ap()
    out = nc.dram_tensor("out", [S, HID], F16, kind="ExternalOutput").ap()

    with tile.TileContext(nc) as tc:
        _kernel(tc, xt, wqkv, wo, cos_d, sinr_d, zmask_d, ones_d, ebias_d,
                id_d, out)
    nc.compile()
    return nc


def _kernel(tc, xt, wqkv, wo, cos_d, sinr_d, zmask_d, ones_d, ebias_d, id_d,
            out):
    nc = tc.nc

    with ExitStack() as big:
        persist = big.enter_context(tc.tile_pool(name="persist", bufs=1))
        ones_sb = persist.tile([P, P], F16)
        ebias_sb = persist.tile([P, 1], F32)
        id_sb = persist.tile([P, P], F16)
        roped = persist.tile([P, QH + 1, S], F16)   # rotated q0..q3, K
        v_nat = persist.tile([P, NST, D], F16)      # V in [seq-tile, dim]
        w_sb = persist.tile([P, NKT, FEAT], F16)    # full w_qkv resident
        wq3 = wqkv.rearrange("(kt p) f -> p kt f", p=P)
        xt3 = xt.rearrange("(kt p) s -> p kt s", p=P)

        # ---- phase 1: qkvT = wqkv^T @ XT, rope chunks interleaved ----
        with tc.tile_pool(name="qkvf", bufs=2) as qkvf_pool, \
             tc.tile_pool(name="cs", bufs=1) as cs_pool, \
             tc.tile_pool(name="rtmp", bufs=2) as rtmp, \
             tc.tile_pool(name="xts", bufs=4) as xt_pool:
            cos_sb = cs_pool.tile([D, S], F16)
            sinr_sb = cs_pool.tile([D, S], F16)

            # startup: first matmul only needs w k-tiles 0-1 + xt chunk 0
            # (~1.6 MB); order the boot DMAs accordingly.
            xts0 = []
            nc.sync.dma_start(out=w_sb[:, 0:2, :], in_=wq3[:, 0:2, :])
            t = xt_pool.tile([P, 4, 512], F16, tag="xt0", name="xtchunk0")
            nc.sync.dma_start(out=t, in_=xt3[:, 0:4, 0:512])
            xts0.append(t)
            nc.sync.dma_start(out=w_sb[:, 2:6, :], in_=wq3[:, 2:6, :])
            for c in range(1, 8):
                t = xt_pool.tile([P, 4, 512], F16, tag="xt0", name="xtchunk0")
                nc.sync.dma_start(out=t, in_=xt3[:, 4 * c:4 * c + 4, 0:512])
                xts0.append(t)
                if c < 7:
                    nc.sync.dma_start(out=w_sb[:, 4 * c + 2:4 * c + 6, :],
                                      in_=wq3[:, 4 * c + 2:4 * c + 6, :])
            nc.sync.dma_start(out=w_sb[:, 30:32, :], in_=wq3[:, 30:32, :])
            nc.sync.dma_start(out=cos_sb, in_=cos_d)
            nc.sync.dma_start(out=sinr_sb, in_=sinr_d)
            nc.sync.dma_start(out=ones_sb, in_=ones_d)
            nc.sync.dma_start(out=ebias_sb, in_=ebias_d)
            nc.sync.dma_start(out=id_sb, in_=id_d)

            with tc.tile_pool(name="qkps", bufs=1, space="PSUM") as qk_ps, \
                 tc.tile_pool(name="tps", bufs=2, space="PSUM") as tp_ps:
                for sw in range(NSW):
                    win = slice(sw * 512, (sw + 1) * 512)
                    qkv = qkvf_pool.tile([P, NM, 512], F16, tag="qkv",
                                         name="qkvwin")
                    ps = [qk_ps.tile([P, 512], F32, tag=f"m{m}",
                                     name=f"qkps{m}") for m in range(NM)]
                    if sw == 0:
                        xts = xts0
                    else:
                        xts = []
                        for c in range(4):
                            t = xt_pool.tile([P, 8, 512], F16, tag="xt",
                                             name="xtchunk")
                            nc.sync.dma_start(
                                out=t, in_=xt3[:, 8 * c:8 * c + 8, win])
                            xts.append(t)
                    csz = 4 if sw == 0 else 8
                    for kt in range(NKT):
                        for m in range(NM):
                            nc.tensor.matmul(
                                ps[m], w_sb[:, kt, m * P:(m + 1) * P],
                                xts[kt // csz][:, kt % csz, :],
                                start=(kt == 0), stop=(kt == NKT - 1))
                    for m in range(NM):
                        nc.scalar.copy(out=qkv[:, m, :], in_=ps[m])
                    for m in [QH] + list(range(QH)):   # rope K, then q0..q3
                        row = qkv[:, m, :]
                        tmp = rtmp.tile([P, 512], F16, tag="rt",
                                        name="ropetmp")
                        nc.vector.tensor_mul(tmp[0:64, :], row[64:128, :],
                                             sinr_sb[64:128, win])
                        nc.vector.tensor_mul(tmp[64:128, :], row[0:64, :],
                                             sinr_sb[0:64, win])
                        nc.vector.tensor_mul(row, row, cos_sb[:, win])
                        nc.vector.tensor_add(roped[:, m, win], row, tmp)
                    # V transpose for this window's 4 seq blocks
                    for st in range(4 * sw, 4 * sw + 4):
                        tp = tp_ps.tile([P, P], F16, tag="tp", name="tpps")
                        nc.tensor.transpose(
                            tp, qkv[:, QH + 1, (st % 4) * P:(st % 4 + 1) * P],
                            id_sb)
                        nc.vector.tensor_copy(out=v_nat[:, st, :], in_=tp)

        # ---- phase 2: attention with o_proj interleaved per k-tile ----
        kt_row = roped[:, QH, :]
        wo3 = wo.rearrange("(ft p) e -> p ft e", p=P)
        with tc.tile_pool(name="atn", bufs=1) as atn_pool, \
             tc.tile_pool(name="wop", bufs=1) as wo_pool, \
             tc.tile_pool(name="mskp", bufs=1) as mask_pool, \
             tc.tile_pool(name="pts", bufs=8) as pt_pool, \
             tc.tile_pool(name="accp", bufs=2) as acc_pool, \
             tc.tile_pool(name="nsc", bufs=2) as norm_sc, \
             tc.tile_pool(name="ost", bufs=2) as o_stage, \
             tc.tile_pool(name="stps", bufs=3, space="PSUM") as st_ps, \
             tc.tile_pool(name="pvps", bufs=2, space="PSUM") as pv_ps, \
             tc.tile_pool(name="ops", bufs=1, space="PSUM") as o_ps:
            attnT = atn_pool.tile([P, QH, S], F16)
            wo_sb = wo_pool.tile([P, QH, HID], F16)
            zmask_sb = mask_pool.tile([P, 4, 512], F16)
            nc.sync.dma_start(out=zmask_sb, in_=zmask_d)
            for f in range(QH):
                nc.sync.dma_start(out=wo_sb[:, f, :], in_=wo3[:, f, :])

            def make_oproj_thunks(st):
                """o_proj for seq-tile st as a list of single-step thunks.
                4 PSUM banks: ecg pairs alternate (o0,o1)/(o2,o3)."""
                ss = slice(st * P, (st + 1) * P)
                state = {}
                thunks = []

                def alloc_stage():
                    state['stg'] = o_stage.tile([P, HID], F16, tag="stg",
                                                name="ostg")
                for ecg in range(4):
                    for f in range(QH):
                        for ec in range(2):
                            def mm(f=f, ec=ec, ecg=ecg):
                                if f == 0 and ec == 0:
                                    if ecg == 0:
                                        alloc_stage()
                                    state[ecg] = [
                                        o_ps.tile([P, 512], F32,
                                                  tag=f"o{(2 * ecg + e) % 3}",
                                                  name=f"ops{(2 * ecg + e) % 3}")
                                        for e in range(2)]
                                c0 = (ecg * 2 + ec) * 512
                                nc.tensor.matmul(
                                    state[ecg][ec], attnT[:, f, ss],
                                    wo_sb[:, f, c0:c0 + 512],
                                    start=(f == 0), stop=(f == QH - 1))
                            thunks.append(mm)
                    for ec in range(2):
                        def stage(ec=ec, ecg=ecg):
                            c0 = (ecg * 2 + ec) * 512
                            dst = state['stg'][:, c0:c0 + 512]
                            if ec == 0:
                                nc.vector.tensor_copy(out=dst,
                                                      in_=state[ecg][ec])
                            else:
                                nc.scalar.copy(out=dst, in_=state[ecg][ec])
                        thunks.append(stage)

                    def dma(ecg=ecg):
                        c0 = ecg * 1024
                        eng = nc.sync if st % 2 == 0 else nc.scalar
                        eng.dma_start(out=out[ss, c0:c0 + 1024],
                                      in_=state['stg'][:, c0:c0 + 1024])
                    thunks.append(dma)
                return thunks

            pending = []          # o_proj thunks of previous windows

            def drain(k):
                for _ in range(min(k, len(pending))):
                    pending.pop(0)()

            for qw in reversed(range(NSW)):   # densest window first (HAM)
                n_kt = 4 * (qw + 1)
                qs = slice(qw * 512, (qw + 1) * 512)
                iters_left = QH * n_kt
                for h in range(QH):
                    qr = roped[:, h, qs]
                    pv = pv_ps.tile([P, 512], F32, tag="pv", name="pvps")
                    acc = acc_pool.tile([P, 512], F16, tag="acc", name="acct")
                    prev = None
                    for kt in range(n_kt):
                        j = kt - 4 * qw
                        c0 = max(j, 0) * P
                        cs_ = slice(c0, 512)
                        stp = st_ps.tile([P, 512], F32, tag="st", name="stps")
                        nc.tensor.matmul(
                            stp[:, cs_], kt_row[:, kt * P:(kt + 1) * P],
                            qr[:, cs_], start=True, stop=True)
                        pt = pt_pool.tile([P, 512], F16, tag="pt",
                                          name="ptile")
                        nc.scalar.activation(out=pt[:, cs_], in_=stp[:, cs_],
                                             func=EXP, scale=SCALE,
                                             bias=ebias_sb)
                        if j >= 0:   # diagonal: zero the masked region
                            nc.vector.tensor_mul(pt[:, cs_], pt[:, cs_],
                                                 zmask_sb[:, j, cs_])
                        if kt == 0:
                            nc.vector.tensor_copy(out=acc, in_=pt)
                        else:
                            nc.vector.tensor_add(acc[:, cs_], acc[:, cs_],
                                                 pt[:, cs_])
                        # PV lags one k-tile so the PE never waits on exp
                        if prev is not None:
                            pk, pp, pcs = prev
                            nc.tensor.matmul(pv[:, pcs], v_nat[:, pk, :],
                                             pp[:, pcs], start=(pk == 0),
                                             stop=False)
                        if pending:
                            drain(-(-len(pending) // iters_left))
                        iters_left -= 1
                        prev = (kt, pt, cs_)
                    pk, pp, pcs = prev
                    nc.tensor.matmul(pv[:, pcs], v_nat[:, pk, :], pp[:, pcs],
                                     start=(pk == 0), stop=True)
                    drain(2)
                    # rowsum broadcast: bc[m, q] = sum_p acc[p, q]
                    bc = st_ps.tile([P, 512], F32, tag="st", name="bcps")
                    nc.tensor.matmul(bc, ones_sb, acc, start=True, stop=True)
                    rec = norm_sc.tile([P, 512], F32, tag="rec", name="recb")
                    nc.vector.reciprocal_approx_fast(out=rec, in_=bc)
                    nc.vector.tensor_mul(attnT[:, h, qs], pv, rec)

                for st in range(4 * qw, 4 * qw + 4):
                    pending.extend(make_oproj_thunks(st))
            drain(len(pending))


def _host_inputs(positions, hidden_states, w_qkv, w_o):
    """Build the 8 per-core input maps (host-side sharding + layout prep)."""
    f16 = np.float16
    x = np.ascontiguousarray(
        hidden_states.reshape(S, HID).T.astype(f16))     # [HID, S] fp16

    pos = positions.reshape(S).astype(np.float32)
    inv = (1.0 / (ROPE_THETA ** (np.arange(0, D, 2, dtype=np.float32) / D)))
    ang = inv[:, None] * pos[None, :]                    # [64, S]
    cos = np.concatenate([np.cos(ang), np.cos(ang)], 0).astype(f16)
    sinr = np.concatenate([np.sin(ang), -np.sin(ang)], 0).astype(f16)

    zmask = np.zeros((P, 4, 512), dtype=f16)
    k_loc = np.arange(P)[:, None]
    q_loc = np.arange(512)[None, :]
    for j in range(4):
        zmask[:, j, :] = (k_loc + P * j <= q_loc).astype(f16)

    ones_sq = np.ones((P, P), f16)
    ebias = np.full((P, 1), ESHIFT, np.float32)
    ident = np.eye(P, dtype=f16)

    in_maps = []
    for i in range(NCORES):
        wq = w_qkv[:, i * FO:(i + 1) * FO]
        wk = w_qkv[:, N_HEADS * D + i * D: N_HEADS * D + (i + 1) * D]
        wv = w_qkv[:, (N_HEADS + N_KV) * D + i * D:
                   (N_HEADS + N_KV) * D + (i + 1) * D]
        wqkv_i = np.ascontiguousarray(
            np.concatenate([wq, wk, wv], axis=1).astype(f16))
        wo_i = np.ascontiguousarray(w_o[i * FO:(i + 1) * FO, :].astype(f16))
        in_maps.append({
            "xt": x, "wqkv": wqkv_i, "wo": wo_i, "cos": cos, "sinr": sinr,
            "zmask": zmask, "ones_sq": ones_sq, "ebias": ebias,
            "ident": ident,
        })
    return in_maps


def kernel(positions, hidden_states, w_qkv, w_o):
    positions = np.asarray(positions)
    hidden_states = np.asarray(hidden_states, dtype=np.float32)
    w_qkv = np.asarray(w_qkv, dtype=np.float32)
    w_o = np.asarray(w_o, dtype=np.float32)

    if "nc" not in _CACHE:
        _CACHE["nc"] = _build_nc()
    nc = _CACHE["nc"]

    in_maps = _host_inputs(positions, hidden_states, w_qkv, w_o)
    res = run_bass_kernel_spmd(nc, in_maps, list(range(NCORES)))
    _CACHE["last_res"] = res

    acc = np.zeros((S, HID), dtype=np.float32)
    for c in range(NCORES):
        acc += res.results[c]["out"].astype(np.float32)
    return acc.reshape(1, S, HID)


# revision 14
# speedup vs baseline: 1.4788x; 1.0085x over previous
"""Mixtral-style GQA attention (B=1, S=2048, HID=4096, 32 q-heads / 8 kv-heads,
head_dim=128, NeoX RoPE, causal) on 8 Trainium2 NeuronCores.

Sharding: tensor-parallel over heads. Core i gets q-heads [4i..4i+3] and
kv-head i (w_qkv columns), plus the matching w_o rows. Each core computes a
full-shape fp16 partial of the output projection; the host sums the 8
partials in fp32 (the "all-reduce") and returns the full output.

Device layout (v4, all-fp16):
 - Every matmul operand is fp16 (fp32r measures ~2 cycles/row on TRN2;
   16-bit streams 1 col/cycle and FWL hides LDWEIGHTS). PSUM stays fp32.
 - Softmax skips max-subtraction but shifts exp by a constant -5 so P and
   its rowsums fit fp16 comfortably (max score*SCALE ~ 10, rowsums < 256;
   the shift cancels exactly in P/rowsum). Causal mask is a 0/1 fp16
   multiply on the exp output, off the score->exp critical path.
 - Rowsums accumulate on DVE in fp16 (2x mode) and broadcast to 128
   partitions with one all-ones [128,128] matmul per head-window; no
   per-k-tile rowsum matmuls on the PE.
 - PV and rowsum consumption lag the score/exp pipeline by one k-tile so
   the in-order PE queue never waits on ACT exp latency.
 - Attention windows run 3,2,1,0 (legal: all K/V ready after phase 1) so
   the first window is the densest and the HAM clock gate never drops.
 - o_proj (previous window) interleaves into the attention k-tile loop at
   micro-op granularity; 3 PSUM banks, staging evictions alternate
   DVE/ACT into a [128, 4096] fp16 tile -> one output DMA per seq-tile.
 - hidden_states passed pre-transposed (XT [HID, S]); w_qkv fully
   SBUF-resident; XT streams per-window in batched [128, 8, 512] DMAs
   issued from the Sync engine.
"""
from contextlib import ExitStack

import numpy as np

import concourse.bacc as bacc
import concourse.tile as tile
from concourse import mybir
from concourse.bass_utils import run_bass_kernel_spmd

# ---- problem constants (hardcoded per contest contract) ----
HID = 4096
S = 2048
N_HEADS = 32
N_KV = 8
D = 128                    # head_dim
NCORES = 8
QH = N_HEADS // NCORES     # 4 q-heads per core
FEAT = QH * D + 2 * D      # 768 per-core qkv output columns (q0..q3, k, v)
FO = QH * D                # 512 per-core attn features for o_proj
ROPE_THETA = 10000.0
SCALE = D ** -0.5
ESHIFT = -5.0              # exp(s*SCALE + ESHIFT); cancels in normalization

P = 128
F32 = mybir.dt.float32
F16 = mybir.dt.float16
EXP = mybir.ActivationFunctionType.Exp

NKT = HID // P     # 32 hidden k-tiles
NSW = S // 512     # 4 seq windows
NM = FEAT // P     # 6 qkv m-tiles
NST = S // P       # 16 seq tiles

_CACHE = {}


def _build_nc():
    nc = bacc.Bacc("TRN2", target_bir_lowering=False, debug=False)

    xt = nc.dram_tensor("xt", [HID, S], F16, kind="ExternalInput").ap()
    wqkv = nc.dram_tensor("wqkv", [HID, FEAT], F16, kind="ExternalInput").ap()
    wo = nc.dram_tensor("wo", [FO, HID], F16, kind="ExternalInput").ap()
    cos_d = nc.dram_tensor("cos", [D, S], F16, kind="ExternalInput").ap()
    sinr_d = nc.dram_tensor("sinr", [D, S], F16, kind="ExternalInput").ap()
    zmask_d = nc.dram_tensor("zmask", [P, 4, 512], F16, kind="ExternalInput").ap()
    ones_d = nc.dram_tensor("ones_sq", [P, P], F16, kind="ExternalInput").ap()
    ebias_d = nc.dram_tensor("ebias", [P, 1], F32, kind="ExternalInput").ap()
    id_d = nc.dram_tensor("ident", [P, P], F16, kind="ExternalInput").ap()
    out = nc.dram_tensor("out", [S, HID], F16, kind="ExternalOutput").ap()

    with tile.TileContext(nc) as tc:
        _kernel(tc, xt, wqkv, wo, cos_d, sinr_d, zmask_d, ones_d, ebias_d,
                id_d, out)
    nc.compile()
    return nc


def _kernel(tc, xt, wqkv, wo, cos_d, sinr_d, zmask_d, ones_d, ebias_d, id_d,
            out):
    nc = tc.nc

    with ExitStack() as big:
        persist = big.enter_context(tc.tile_pool(name="persist", bufs=1))
        ones_sb = persist.tile([P, P], F16)
        ebias_sb = persist.tile([P, 1], F32)
        id_sb = persist.tile([P, P], F16)
        roped = persist.tile([P, QH + 1, S], F16)   # rotated q0..q3, K
        v_nat = persist.tile([P, NST, D], F16)      # V in [seq-tile, dim]
        w_sb = persist.tile([P, NKT, FEAT], F16)    # full w_qkv resident
        wq3 = wqkv.rearrange("(kt p) f -> p kt f", p=P)
        xt3 = xt.rearrange("(kt p) s -> p kt s", p=P)

        # ---- phase 1: qkvT = wqkv^T @ XT, rope chunks interleaved ----
        with tc.tile_pool(name="qkvf", bufs=2) as qkvf_pool, \
             tc.tile_pool(name="cs", bufs=1) as cs_pool, \
             tc.tile_pool(name="rtmp", bufs=2) as rtmp, \
             tc.tile_pool(name="xts", bufs=4) as xt_pool:
            cos_sb = cs_pool.tile([D, S], F16)
            sinr_sb = cs_pool.tile([D, S], F16)

            # startup: first matmul only needs w k-tiles 0-1 + xt chunk 0
            # (~1.6 MB); order the boot DMAs accordingly.
            xts0 = []
            nc.sync.dma_start(out=w_sb[:, 0:2, :], in_=wq3[:, 0:2, :])
            t = xt_pool.tile([P, 4, 512], F16, tag="xt0", name="xtchunk0")
            nc.sync.dma_start(out=t, in_=xt3[:, 0:4, 0:512])
            xts0.append(t)
            nc.sync.dma_start(out=w_sb[:, 2:6, :], in_=wq3[:, 2:6, :])
            for c in range(1, 8):
                t = xt_pool.tile([P, 4, 512], F16, tag="xt0", name="xtchunk0")
                nc.sync.dma_start(out=t, in_=xt3[:, 4 * c:4 * c + 4, 0:512])
                xts0.append(t)
                if c < 7:
                    nc.sync.dma_start(out=w_sb[:, 4 * c + 2:4 * c + 6, :],
                                      in_=wq3[:, 4 * c + 2:4 * c + 6, :])
            nc.sync.dma_start(out=w_sb[:, 30:32, :], in_=wq3[:, 30:32, :])
            nc.sync.dma_start(out=cos_sb, in_=cos_d)
            nc.sync.dma_start(out=sinr_sb, in_=sinr_d)
            nc.sync.dma_start(out=ones_sb, in_=ones_d)
            nc.sync.dma_start(out=ebias_sb, in_=ebias_d)
            nc.sync.dma_start(out=id_sb, in_=id_d)

            with tc.tile_pool(name="qkps", bufs=1, space="PSUM") as qk_ps, \
                 tc.tile_pool(name="tps", bufs=2, space="PSUM") as tp_ps:
                for sw in range(NSW):
                    win = slice(sw * 512, (sw + 1) * 512)
                    qkv = qkvf_pool.tile([P, NM, 512], F16, tag="qkv",
                                         name="qkvwin")
                    ps = [qk_ps.tile([P, 512], F32, tag=f"m{m}",
                                     name=f"qkps{m}") for m in range(NM)]
                    if sw == 0:
                        xts = xts0
                    else:
                        xts = []
                        for c in range(4):
                            t = xt_pool.tile([P, 8, 512], F16, tag="xt",
                                             name="xtchunk")
                            nc.sync.dma_start(
                                out=t, in_=xt3[:, 8 * c:8 * c + 8, win])
                            xts.append(t)
                    csz = 4 if sw == 0 else 8
                    for kt in range(NKT):
                        for m in range(NM):
                            nc.tensor.matmul(
                                ps[m], w_sb[:, kt, m * P:(m + 1) * P],
                                xts[kt // csz][:, kt % csz, :],
                                start=(kt == 0), stop=(kt == NKT - 1))
                    for m in range(NM):
                        if m % 2 == 0:
                            nc.scalar.copy(out=qkv[:, m, :], in_=ps[m])
                        else:
                            nc.vector.tensor_copy(out=qkv[:, m, :],
                                                  in_=ps[m])
                    for m in [QH] + list(range(QH)):   # rope K, then q0..q3
                        row = qkv[:, m, :]
                        tmp = rtmp.tile([P, 512], F16, tag="rt",
                                        name="ropetmp")
                        nc.vector.tensor_mul(tmp[0:64, :], row[64:128, :],
                                             sinr_sb[64:128, win])
                        nc.vector.tensor_mul(tmp[64:128, :], row[0:64, :],
                                             sinr_sb[0:64, win])
                        nc.vector.tensor_mul(row, row, cos_sb[:, win])
                        nc.vector.tensor_add(roped[:, m, win], row, tmp)
                    # V transpose for this window's 4 seq blocks
                    for st in range(4 * sw, 4 * sw + 4):
                        tp = tp_ps.tile([P, P], F16, tag="tp", name="tpps")
                        nc.tensor.transpose(
                            tp, qkv[:, QH + 1, (st % 4) * P:(st % 4 + 1) * P],
                            id_sb)
                        nc.vector.tensor_copy(out=v_nat[:, st, :], in_=tp)

        # ---- phase 2: attention with o_proj interleaved per k-tile ----
        kt_row = roped[:, QH, :]
        wo3 = wo.rearrange("(ft p) e -> p ft e", p=P)
        with tc.tile_pool(name="atn", bufs=1) as atn_pool, \
             tc.tile_pool(name="wop", bufs=1) as wo_pool, \
             tc.tile_pool(name="mskp", bufs=1) as mask_pool, \
             tc.tile_pool(name="pts", bufs=8) as pt_pool, \
             tc.tile_pool(name="accp", bufs=2) as acc_pool, \
             tc.tile_pool(name="nsc", bufs=2) as norm_sc, \
             tc.tile_pool(name="ost", bufs=2) as o_stage, \
             tc.tile_pool(name="stps", bufs=3, space="PSUM") as st_ps, \
             tc.tile_pool(name="pvps", bufs=2, space="PSUM") as pv_ps, \
             tc.tile_pool(name="ops", bufs=1, space="PSUM") as o_ps:
            attnT = atn_pool.tile([P, QH, S], F16)
            wo_sb = wo_pool.tile([P, QH, HID], F16)
            zmask_sb = mask_pool.tile([P, 4, 512], F16)
            nc.sync.dma_start(out=zmask_sb, in_=zmask_d)
            for f in range(QH):
                nc.sync.dma_start(out=wo_sb[:, f, :], in_=wo3[:, f, :])

            def make_oproj_thunks(st):
                """o_proj for seq-tile st as a list of single-step thunks.
                4 PSUM banks: ecg pairs alternate (o0,o1)/(o2,o3)."""
                ss = slice(st * P, (st + 1) * P)
                state = {}
                thunks = []

                def alloc_stage():
                    state['stg'] = o_stage.tile([P, HID], F16, tag="stg",
                                                name="ostg")
                for ecg in range(4):
                    for f in range(QH):
                        for ec in range(2):
                            def mm(f=f, ec=ec, ecg=ecg):
                                if f == 0 and ec == 0:
                                    if ecg == 0:
                                        alloc_stage()
                                    state[ecg] = [
                                        o_ps.tile([P, 512], F32,
                                                  tag=f"o{(2 * ecg + e) % 3}",
                                                  name=f"ops{(2 * ecg + e) % 3}")
                                        for e in range(2)]
                                c0 = (ecg * 2 + ec) * 512
                                nc.tensor.matmul(
                                    state[ecg][ec], attnT[:, f, ss],
                                    wo_sb[:, f, c0:c0 + 512],
                                    start=(f == 0), stop=(f == QH - 1))
                            thunks.append(mm)
                    for ec in range(2):
                        def stage(ec=ec, ecg=ecg):
                            c0 = (ecg * 2 + ec) * 512
                            dst = state['stg'][:, c0:c0 + 512]
                            if ec == 0:
                                nc.vector.tensor_copy(out=dst,
                                                      in_=state[ecg][ec])
                            else:
                                nc.scalar.copy(out=dst, in_=state[ecg][ec])
                        thunks.append(stage)

                    def dma(ecg=ecg):
                        c0 = ecg * 1024
                        eng = nc.sync if st % 2 == 0 else nc.scalar
                        eng.dma_start(out=out[ss, c0:c0 + 1024],
                                      in_=state['stg'][:, c0:c0 + 1024])
                    thunks.append(dma)
                return thunks

            pending = []          # o_proj thunks of previous windows

            def drain(k):
                for _ in range(min(k, len(pending))):
                    pending.pop(0)()

            for qw in [2, 3, 1, 0]:   # start on long-ready inputs (HAM)
                n_kt = 4 * (qw + 1)
                qs = slice(qw * 512, (qw + 1) * 512)
                iters_left = QH * n_kt
                for h in range(QH):
                    qr = roped[:, h, qs]
                    pv = pv_ps.tile([P, 512], F32, tag="pv", name="pvps")
                    acc = acc_pool.tile([P, 512], F16, tag="acc", name="acct")
                    prev = None
                    for kt in range(n_kt):
                        j = kt - 4 * qw
                        c0 = max(j, 0) * P
                        cs_ = slice(c0, 512)
                        stp = st_ps.tile([P, 512], F32, tag="st", name="stps")
                        nc.tensor.matmul(
                            stp[:, cs_], kt_row[:, kt * P:(kt + 1) * P],
                            qr[:, cs_], start=True, stop=True)
                        pt = pt_pool.tile([P, 512], F16, tag="pt",
                                          name="ptile")
                        nc.scalar.activation(out=pt[:, cs_], in_=stp[:, cs_],
                                             func=EXP, scale=SCALE,
                                             bias=ebias_sb)
                        if j >= 0:   # diagonal: zero the masked region
                            nc.vector.tensor_mul(pt[:, cs_], pt[:, cs_],
                                                 zmask_sb[:, j, cs_])
                        if kt == 0:
                            nc.vector.tensor_copy(out=acc, in_=pt)
                        else:
                            nc.vector.tensor_add(acc[:, cs_], acc[:, cs_],
                                                 pt[:, cs_])
                        # PV lags one k-tile so the PE never waits on exp
                        if prev is not None:
                            pk, pp, pcs = prev
                            nc.tensor.matmul(pv[:, pcs], v_nat[:, pk, :],
                                             pp[:, pcs], start=(pk == 0),
                                             stop=False)
                        if pending:
                            drain(-(-len(pending) // iters_left))
                        iters_left -= 1
                        prev = (kt, pt, cs_)
                    pk, pp, pcs = prev
                    nc.tensor.matmul(pv[:, pcs], v_nat[:, pk, :], pp[:, pcs],
                                     start=(pk == 0), stop=True)
                    drain(2)
                    # rowsum broadcast: bc[m, q] = sum_p acc[p, q]
                    bc = st_ps.tile([P, 512], F32, tag="st", name="bcps")
                    nc.tensor.matmul(bc, ones_sb, acc, start=True, stop=True)
                    rec = norm_sc.tile([P, 512], F32, tag="rec", name="recb")
                    nc.vector.reciprocal_approx_fast(out=rec, in_=bc)
                    nc.vector.tensor_mul(attnT[:, h, qs], pv, rec)

                for st in range(4 * qw, 4 * qw + 4):
                    pending.extend(make_oproj_thunks(st))
            drain(len(pending))


def _host_inputs(positions, hidden_states, w_qkv, w_o):
    """Build the 8 per-core input maps (host-side sharding + layout prep)."""
    f16 = np.float16
    x = np.ascontiguousarray(
        hidden_states.reshape(S, HID).T.astype(f16))     # [HID, S] fp16

    pos = positions.reshape(S).astype(np.float32)
    inv = (1.0 / (ROPE_THETA ** (np.arange(0, D, 2, dtype=np.float32) / D)))
    ang = inv[:, None] * pos[None, :]                    # [64, S]
    cos = np.concatenate([np.cos(ang), np.cos(ang)], 0).astype(f16)
    sinr = np.concatenate([np.sin(ang), -np.sin(ang)], 0).astype(f16)

    zmask = np.zeros((P, 4, 512), dtype=f16)
    k_loc = np.arange(P)[:, None]
    q_loc = np.arange(512)[None, :]
    for j in range(4):
        zmask[:, j, :] = (k_loc + P * j <= q_loc).astype(f16)

    ones_sq = np.ones((P, P), f16)
    ebias = np.full((P, 1), ESHIFT, np.float32)
    ident = np.eye(P, dtype=f16)

    in_maps = []
    for i in range(NCORES):
        wq = w_qkv[:, i * FO:(i + 1) * FO]
        wk = w_qkv[:, N_HEADS * D + i * D: N_HEADS * D + (i + 1) * D]
        wv = w_qkv[:, (N_HEADS + N_KV) * D + i * D:
                   (N_HEADS + N_KV) * D + (i + 1) * D]
        wqkv_i = np.ascontiguousarray(
            np.concatenate([wq, wk, wv], axis=1).astype(f16))
        wo_i = np.ascontiguousarray(w_o[i * FO:(i + 1) * FO, :].astype(f16))
        in_maps.append({
            "xt": x, "wqkv": wqkv_i, "wo": wo_i, "cos": cos, "sinr": sinr,
            "zmask": zmask, "ones_sq": ones_sq, "ebias": ebias,
            "ident": ident,
        })
    return in_maps


def kernel(positions, hidden_states, w_qkv, w_o):
    positions = np.asarray(positions)
    hidden_states = np.asarray(hidden_states, dtype=np.float32)
    w_qkv = np.asarray(w_qkv, dtype=np.float32)
    w_o = np.asarray(w_o, dtype=np.float32)

    if "nc" not in _CACHE:
        _CACHE["nc"] = _build_nc()
    nc = _CACHE["nc"]

    in_maps = _host_inputs(positions, hidden_states, w_qkv, w_o)
    res = run_bass_kernel_spmd(nc, in_maps, list(range(NCORES)))
    _CACHE["last_res"] = res

    acc = np.zeros((S, HID), dtype=np.float32)
    for c in range(NCORES):
        acc += res.results[c]["out"].astype(np.float32)
    return acc.reshape(1, S, HID)


# revision 15
# speedup vs baseline: 1.4937x; 1.0101x over previous
"""Mixtral-style GQA attention (B=1, S=2048, HID=4096, 32 q-heads / 8 kv-heads,
head_dim=128, NeoX RoPE, causal) on 8 Trainium2 NeuronCores.

Sharding: tensor-parallel over heads. Core i gets q-heads [4i..4i+3] and
kv-head i (w_qkv columns), plus the matching w_o rows. Each core computes a
full-shape fp16 partial of the output projection; the host sums the 8
partials in fp32 (the "all-reduce") and returns the full output.

Device layout (v4, all-fp16):
 - Every matmul operand is fp16 (fp32r measures ~2 cycles/row on TRN2;
   16-bit streams 1 col/cycle and FWL hides LDWEIGHTS). PSUM stays fp32.
 - Softmax skips max-subtraction but shifts exp by a constant -5 so P and
   its rowsums fit fp16 comfortably (max score*SCALE ~ 10, rowsums < 256;
   the shift cancels exactly in P/rowsum). Causal mask is a 0/1 fp16
   multiply on the exp output, off the score->exp critical path.
 - Rowsums accumulate on DVE in fp16 (2x mode) and broadcast to 128
   partitions with one all-ones [128,128] matmul per head-window; no
   per-k-tile rowsum matmuls on the PE.
 - PV and rowsum consumption lag the score/exp pipeline by one k-tile so
   the in-order PE queue never waits on ACT exp latency.
 - Attention windows run 3,2,1,0 (legal: all K/V ready after phase 1) so
   the first window is the densest and the HAM clock gate never drops.
 - o_proj (previous window) interleaves into the attention k-tile loop at
   micro-op granularity; 3 PSUM banks, staging evictions alternate
   DVE/ACT into a [128, 4096] fp16 tile -> one output DMA per seq-tile.
 - hidden_states passed pre-transposed (XT [HID, S]); w_qkv fully
   SBUF-resident; XT streams per-window in batched [128, 8, 512] DMAs
   issued from the Sync engine.
"""
from contextlib import ExitStack

import numpy as np

import concourse.bacc as bacc
import concourse.tile as tile
from concourse import mybir
from concourse.bass_utils import run_bass_kernel_spmd

# ---- problem constants (hardcoded per contest contract) ----
HID = 4096
S = 2048
N_HEADS = 32
N_KV = 8
D = 128                    # head_dim
NCORES = 8
QH = N_HEADS // NCORES     # 4 q-heads per core
FEAT = QH * D + 2 * D      # 768 per-core qkv output columns (q0..q3, k, v)
FO = QH * D                # 512 per-core attn features for o_proj
ROPE_THETA = 10000.0
SCALE = D ** -0.5
ESHIFT = -5.0              # exp(s*SCALE + ESHIFT); cancels in normalization

P = 128
F32 = mybir.dt.float32
F16 = mybir.dt.float16
EXP = mybir.ActivationFunctionType.Exp

NKT = HID // P     # 32 hidden k-tiles
NSW = S // 512     # 4 seq windows
NM = FEAT // P     # 6 qkv m-tiles
NST = S // P       # 16 seq tiles

_CACHE = {}


def _build_nc():
    nc = bacc.Bacc("TRN2", target_bir_lowering=False, debug=False)

    xt = nc.dram_tensor("xt", [HID, S], F16, kind="ExternalInput").ap()
    wqkv = nc.dram_tensor("wqkv", [HID, FEAT], F16, kind="ExternalInput").ap()
    wo = nc.dram_tensor("wo", [FO, HID], F16, kind="ExternalInput").ap()
    cos_d = nc.dram_tensor("cos", [D, S], F16, kind="ExternalInput").ap()
    sinr_d = nc.dram_tensor("sinr", [D, S], F16, kind="ExternalInput").ap()
    zmask_d = nc.dram_tensor("zmask", [P, 4, 512], F16, kind="ExternalInput").ap()
    ones_d = nc.dram_tensor("ones_sq", [P, P], F16, kind="ExternalInput").ap()
    ebias_d = nc.dram_tensor("ebias", [P, 1], F32, kind="ExternalInput").ap()
    id_d = nc.dram_tensor("ident", [P, P], F16, kind="ExternalInput").ap()
    out = nc.dram_tensor("out", [S, HID], F16, kind="ExternalOutput").ap()

    with tile.TileContext(nc) as tc:
        _kernel(tc, xt, wqkv, wo, cos_d, sinr_d, zmask_d, ones_d, ebias_d,
                id_d, out)
    nc.compile()
    return nc


def _kernel(tc, xt, wqkv, wo, cos_d, sinr_d, zmask_d, ones_d, ebias_d, id_d,
            out):
    nc = tc.nc

    with ExitStack() as big:
        persist = big.enter_context(tc.tile_pool(name="persist", bufs=1))
        ones_sb = persist.tile([P, P], F16)
        ebias_sb = persist.tile([P, 1], F32)
        id_sb = persist.tile([P, P], F16)
        roped = persist.tile([P, QH + 1, S], F16)   # rotated q0..q3, K
        v_nat = persist.tile([P, NST, D], F16)      # V in [seq-tile, dim]
        w_sb = persist.tile([P, NKT, FEAT], F16)    # full w_qkv resident
        wq3 = wqkv.rearrange("(kt p) f -> p kt f", p=P)
        xt3 = xt.rearrange("(kt p) s -> p kt s", p=P)

        # ---- phase 1: qkvT = wqkv^T @ XT, rope chunks interleaved ----
        with tc.tile_pool(name="qkvf", bufs=2) as qkvf_pool, \
             tc.tile_pool(name="cs", bufs=1) as cs_pool, \
             tc.tile_pool(name="rtmp", bufs=2) as rtmp, \
             tc.tile_pool(name="xts", bufs=4) as xt_pool:
            cos_sb = cs_pool.tile([D, S], F16)
            sinr_sb = cs_pool.tile([D, S], F16)

            # startup: first matmul only needs w k-tiles 0-1 + xt chunk 0
            # (~1.6 MB); order the boot DMAs accordingly.
            xts0 = []
            nc.sync.dma_start(out=w_sb[:, 0:2, :], in_=wq3[:, 0:2, :])
            t = xt_pool.tile([P, 4, 512], F16, tag="xt0", name="xtchunk0")
            nc.sync.dma_start(out=t, in_=xt3[:, 0:4, 0:512])
            xts0.append(t)
            nc.sync.dma_start(out=w_sb[:, 2:6, :], in_=wq3[:, 2:6, :])
            for c in range(1, 8):
                t = xt_pool.tile([P, 4, 512], F16, tag="xt0", name="xtchunk0")
                nc.sync.dma_start(out=t, in_=xt3[:, 4 * c:4 * c + 4, 0:512])
                xts0.append(t)
                if c < 7:
                    nc.sync.dma_start(out=w_sb[:, 4 * c + 2:4 * c + 6, :],
                                      in_=wq3[:, 4 * c + 2:4 * c + 6, :])
            nc.sync.dma_start(out=w_sb[:, 30:32, :], in_=wq3[:, 30:32, :])
            nc.sync.dma_start(out=cos_sb, in_=cos_d)
            nc.sync.dma_start(out=sinr_sb, in_=sinr_d)
            nc.sync.dma_start(out=ones_sb, in_=ones_d)
            nc.sync.dma_start(out=ebias_sb, in_=ebias_d)
            nc.sync.dma_start(out=id_sb, in_=id_d)

            with tc.tile_pool(name="qkps", bufs=1, space="PSUM") as qk_ps, \
                 tc.tile_pool(name="tps", bufs=2, space="PSUM") as tp_ps:
                for sw in range(NSW):
                    win = slice(sw * 512, (sw + 1) * 512)
                    qkv = qkvf_pool.tile([P, NM, 512], F16, tag="qkv",
                                         name="qkvwin")
                    ps = [qk_ps.tile([P, 512], F32, tag=f"m{m}",
                                     name=f"qkps{m}") for m in range(NM)]
                    if sw == 0:
                        xts = xts0
                    else:
                        xts = []
                        for c in range(4):
                            t = xt_pool.tile([P, 8, 512], F16, tag="xt",
                                             name="xtchunk")
                            nc.sync.dma_start(
                                out=t, in_=xt3[:, 8 * c:8 * c + 8, win])
                            xts.append(t)
                    csz = 4 if sw == 0 else 8
                    for kt in range(NKT):
                        for m in range(NM):
                            nc.tensor.matmul(
                                ps[m], w_sb[:, kt, m * P:(m + 1) * P],
                                xts[kt // csz][:, kt % csz, :],
                                start=(kt == 0), stop=(kt == NKT - 1))
                    for m in range(NM):
                        if m % 2 == 0:
                            nc.scalar.copy(out=qkv[:, m, :], in_=ps[m])
                        else:
                            nc.vector.tensor_copy(out=qkv[:, m, :],
                                                  in_=ps[m])
                    for m in [QH] + list(range(QH)):   # rope K, then q0..q3
                        row = qkv[:, m, :]
                        tmp = rtmp.tile([P, 512], F16, tag="rt",
                                        name="ropetmp")
                        nc.vector.tensor_mul(tmp[0:64, :], row[64:128, :],
                                             sinr_sb[64:128, win])
                        nc.vector.tensor_mul(tmp[64:128, :], row[0:64, :],
                                             sinr_sb[0:64, win])
                        nc.vector.tensor_mul(row, row, cos_sb[:, win])
                        nc.vector.tensor_add(roped[:, m, win], row, tmp)
                    # V transpose for this window's 4 seq blocks
                    for st in range(4 * sw, 4 * sw + 4):
                        tp = tp_ps.tile([P, P], F16, tag="tp", name="tpps")
                        nc.tensor.transpose(
                            tp, qkv[:, QH + 1, (st % 4) * P:(st % 4 + 1) * P],
                            id_sb)
                        nc.vector.tensor_copy(out=v_nat[:, st, :], in_=tp)

        # ---- phase 2: attention with o_proj interleaved per k-tile ----
        kt_row = roped[:, QH, :]
        wo3 = wo.rearrange("(ft p) e -> p ft e", p=P)
        with tc.tile_pool(name="atn", bufs=1) as atn_pool, \
             tc.tile_pool(name="wop", bufs=1) as wo_pool, \
             tc.tile_pool(name="mskp", bufs=1) as mask_pool, \
             tc.tile_pool(name="pts", bufs=8) as pt_pool, \
             tc.tile_pool(name="accp", bufs=2) as acc_pool, \
             tc.tile_pool(name="nsc", bufs=2) as norm_sc, \
             tc.tile_pool(name="ost", bufs=2) as o_stage, \
             tc.tile_pool(name="stps", bufs=3, space="PSUM") as st_ps, \
             tc.tile_pool(name="pvps", bufs=2, space="PSUM") as pv_ps, \
             tc.tile_pool(name="ops", bufs=1, space="PSUM") as o_ps:
            attnT = atn_pool.tile([P, QH, S], F16)
            wo_sb = wo_pool.tile([P, QH, HID], F16)
            zmask_sb = mask_pool.tile([P, 4, 512], F16)
            nc.sync.dma_start(out=zmask_sb, in_=zmask_d)
            for f in range(QH):
                nc.sync.dma_start(out=wo_sb[:, f, :], in_=wo3[:, f, :])

            def make_oproj_thunks(st):
                """o_proj for seq-tile st as a list of single-step thunks.
                4 PSUM banks: ecg pairs alternate (o0,o1)/(o2,o3)."""
                ss = slice(st * P, (st + 1) * P)
                state = {}
                thunks = []

                def alloc_stage():
                    state['stg'] = o_stage.tile([P, HID], F16, tag="stg",
                                                name="ostg")
                for ecg in range(4):
                    for f in range(QH):
                        for ec in range(2):
                            def mm(f=f, ec=ec, ecg=ecg):
                                if f == 0 and ec == 0:
                                    if ecg == 0:
                                        alloc_stage()
                                    state[ecg] = [
                                        o_ps.tile([P, 512], F32,
                                                  tag=f"o{(2 * ecg + e) % 3}",
                                                  name=f"ops{(2 * ecg + e) % 3}")
                                        for e in range(2)]
                                c0 = (ecg * 2 + ec) * 512
                                nc.tensor.matmul(
                                    state[ecg][ec], attnT[:, f, ss],
                                    wo_sb[:, f, c0:c0 + 512],
                                    start=(f == 0), stop=(f == QH - 1))
                            thunks.append(mm)
                    for ec in range(2):
                        def stage(ec=ec, ecg=ecg):
                            c0 = (ecg * 2 + ec) * 512
                            dst = state['stg'][:, c0:c0 + 512]
                            if ec == 0:
                                nc.vector.tensor_copy(out=dst,
                                                      in_=state[ecg][ec])
                            else:
                                nc.scalar.copy(out=dst, in_=state[ecg][ec])
                        thunks.append(stage)

                    def dma(ecg=ecg):
                        c0 = ecg * 1024
                        eng = nc.sync if st % 2 == 0 else nc.scalar
                        eng.dma_start(out=out[ss, c0:c0 + 1024],
                                      in_=state['stg'][:, c0:c0 + 1024])
                    thunks.append(dma)
                return thunks

            pending = []          # o_proj thunks of previous windows

            def drain(k):
                for _ in range(min(k, len(pending))):
                    pending.pop(0)()

            for qw in [0, 3, 2, 1]:   # smallest window first: its lack of
                                      # o_proj filler leaves PE sparse only
                                      # ~10us (HAM); big windows get filler
                n_kt = 4 * (qw + 1)
                qs = slice(qw * 512, (qw + 1) * 512)
                iters_left = QH * n_kt
                for h in range(QH):
                    qr = roped[:, h, qs]
                    pv = pv_ps.tile([P, 512], F32, tag="pv", name="pvps")
                    acc = acc_pool.tile([P, 512], F16, tag="acc", name="acct")
                    prev = None
                    for kt in range(n_kt):
                        j = kt - 4 * qw
                        c0 = max(j, 0) * P
                        cs_ = slice(c0, 512)
                        stp = st_ps.tile([P, 512], F32, tag="st", name="stps")
                        nc.tensor.matmul(
                            stp[:, cs_], kt_row[:, kt * P:(kt + 1) * P],
                            qr[:, cs_], start=True, stop=True)
                        pt = pt_pool.tile([P, 512], F16, tag="pt",
                                          name="ptile")
                        nc.scalar.activation(out=pt[:, cs_], in_=stp[:, cs_],
                                             func=EXP, scale=SCALE,
                                             bias=ebias_sb)
                        if j >= 0:   # diagonal: zero the masked region
                            nc.vector.tensor_mul(pt[:, cs_], pt[:, cs_],
                                                 zmask_sb[:, j, cs_])
                        if kt == 0:
                            nc.vector.tensor_copy(out=acc, in_=pt)
                        else:
                            nc.vector.tensor_add(acc[:, cs_], acc[:, cs_],
                                                 pt[:, cs_])
                        # PV lags one k-tile so the PE never waits on exp
                        if prev is not None:
                            pk, pp, pcs = prev
                            nc.tensor.matmul(pv[:, pcs], v_nat[:, pk, :],
                                             pp[:, pcs], start=(pk == 0),
                                             stop=False)
                        if pending:
                            drain(-(-len(pending) // iters_left))
                        iters_left -= 1
                        prev = (kt, pt, cs_)
                    pk, pp, pcs = prev
                    nc.tensor.matmul(pv[:, pcs], v_nat[:, pk, :], pp[:, pcs],
                                     start=(pk == 0), stop=True)
                    drain(4)
                    # rowsum broadcast: bc[m, q] = sum_p acc[p, q]
                    bc = st_ps.tile([P, 512], F32, tag="st", name="bcps")
                    nc.tensor.matmul(bc, ones_sb, acc, start=True, stop=True)
                    rec = norm_sc.tile([P, 512], F32, tag="rec", name="recb")
                    nc.vector.reciprocal_approx_fast(out=rec, in_=bc)
                    nc.vector.tensor_mul(attnT[:, h, qs], pv, rec)

                for st in range(4 * qw, 4 * qw + 4):
                    pending.extend(make_oproj_thunks(st))
            drain(len(pending))


def _host_inputs(positions, hidden_states, w_qkv, w_o):
    """Build the 8 per-core input maps (host-side sharding + layout prep)."""
    f16 = np.float16
    x = np.ascontiguousarray(
        hidden_states.reshape(S, HID).T.astype(f16))     # [HID, S] fp16

    pos = positions.reshape(S).astype(np.float32)
    inv = (1.0 / (ROPE_THETA ** (np.arange(0, D, 2, dtype=np.float32) / D)))
    ang = inv[:, None] * pos[None, :]                    # [64, S]
    cos = np.concatenate([np.cos(ang), np.cos(ang)], 0).astype(f16)
    sinr = np.concatenate([np.sin(ang), -np.sin(ang)], 0).astype(f16)

    zmask = np.zeros((P, 4, 512), dtype=f16)
    k_loc = np.arange(P)[:, None]
    q_loc = np.arange(512)[None, :]
    for j in range(4):
        zmask[:, j, :] = (k_loc + P * j <= q_loc).astype(f16)

    ones_sq = np.ones((P, P), f16)
    ebias = np.full((P, 1), ESHIFT, np.float32)
    ident = np.eye(P, dtype=f16)

    in_maps = []
    for i in range(NCORES):
        wq = w_qkv[:, i * FO:(i + 1) * FO]
        wk = w_qkv[:, N_HEADS * D + i * D: N_HEADS * D + (i + 1) * D]
        wv = w_qkv[:, (N_HEADS + N_KV) * D + i * D:
                   (N_HEADS + N_KV) * D + (i + 1) * D]
        wqkv_i = np.ascontiguousarray(
            np.concatenate([wq, wk, wv], axis=1).astype(f16))
        wo_i = np.ascontiguousarray(w_o[i * FO:(i + 1) * FO, :].astype(f16))
        in_maps.append({
            "xt": x, "wqkv": wqkv_i, "wo": wo_i, "cos": cos, "sinr": sinr,
            "zmask": zmask, "ones_sq": ones_sq, "ebias": ebias,
            "ident": ident,
        })
    return in_maps


def kernel(positions, hidden_states, w_qkv, w_o):
    positions = np.asarray(positions)
    hidden_states = np.asarray(hidden_states, dtype=np.float32)
    w_qkv = np.asarray(w_qkv, dtype=np.float32)
    w_o = np.asarray(w_o, dtype=np.float32)

    if "nc" not in _CACHE:
        _CACHE["nc"] = _build_nc()
    nc = _CACHE["nc"]

    in_maps = _host_inputs(positions, hidden_states, w_qkv, w_o)
    res = run_bass_kernel_spmd(nc, in_maps, list(range(NCORES)))
    _CACHE["last_res"] = res

    acc = np.zeros((S, HID), dtype=np.float32)
    for c in range(NCORES):
        acc += res.results[c]["out"].astype(np.float32)
    return acc.reshape(1, S, HID)
